# revision 5
# baseline (speedup 1.0000x reference)
"""Trainium2 Bass kernel for CompanySpecificHeads (MoE-style routed MLP heads).

Semantics (matching the reference):
    out[b] = gelu(z[b] @ W1[cid[b]] + b1[cid[b]]) @ W2[cid[b]] + b2[cid[b]]

Strategy: expert-parallel across 8 NeuronCores, 8 companies per core.
The run is DMA-bound on the W1 stream (HBM cap ~358 GB/s per core), so W1
is carried in fp8 E3M4 (scaled x64 on host; the x1/64 descale is folded
into the gelu activation's `scale` operand) while z / b1-selector / W2 /
gelu output stay fp16 -> the PE runs mixed fp8xfp16 matmuls (verified
numerically exact on HW). This halves the dominant DMA stream vs fp16.

Token capacity is variable per company slot: companies are sorted by
token count (descending) and slot s takes ranks [8s, 8s+8) across the 8
cores, so every core's slot-s company has a similar count and the shared
SPMD program pads only to the group max (~545 padded tokens/core vs 896
with a uniform capacity). Per company c (tokens on the free axis,
capacity tw, h on partitions):
      psum[h, t] = sum_d W1[c][d, h] * zT[c][d, t]     (PE, fp8 x fp16)
      b1*64 folded in with a K=4 fp16 "selector" matmul
  Gelu: ACT engine, psum*(1/64) -> gelu -> SBUF fp16.
  Layer 2: psum2[1, t] += W2[c][hj]^T @ gelu_h[hj, t]  (8 K=128 matmuls)
Host scatters back to [B, 1] and adds b2 (exact, fp32).

DMA discipline (as measured in the fp16 baseline): every DMACopy <=1
sync wait: loads go to fresh SBUF slots, <=8 DMAs per DGE flavor, and
per-company outputs are staged into one SBUF tile and stored at the end.
"""

import numpy as np

B, C, D, H = 4096, 64, 512, 1024
NCORES = 8
CPC = C // NCORES  # companies (slots) per core
KC = D // 128      # contraction chunks of 128
HC = H // 128      # h chunks of 128
WSCALE = 64.0      # W1 fp8 scale (descaled in the gelu activation)
F8MAX = 15.5       # e3m4 max finite

_COMPILED = {}


def _build(caps):
    """Build the Bass/Tile program for per-slot token capacities `caps`."""
    import concourse.bass as bass
    import concourse.bacc as bacc
    import concourse.mybir as mybir
    from concourse.tile import TileContext
    from contextlib import ExitStack

    f32 = mybir.dt.float32
    f16 = mybir.dt.float16
    f8 = mybir.dt.float8e3

    zoff = [0]
    for cap in caps:
        zoff.append(zoff[-1] + KC * cap)
    ZCOLS = zoff[-1]
    ooff = [0]
    for cap in caps:
        ooff.append(ooff[-1] + cap)
    OUTCOLS = ooff[-1]

    SELW = KC * 128          # selector columns (built at max subtile width)
    B1W = CPC * 2 * 128      # b1 columns

    nc = bacc.Bacc(None, target_bir_lowering=False)

    # zt is stored partition-major: zt[p, (s, k, t)] = z[token_t, 128k+p]
    zt_d = nc.dram_tensor("zt", [128, ZCOLS], f16, kind="ExternalInput")
    # w1[s][p][g][k][hh] = W1[comp(s)][128k+p, 512g+hh] * 64, fp8: a whole
    # company loads linearly with 4KB contiguous per partition.
    w1_d = nc.dram_tensor("w1", [CPC, 128, 2, KC, H // 2], f8, kind="ExternalInput")
    cst_d = nc.dram_tensor("cst", [KC, SELW + B1W], f16, kind="ExternalInput")
    w2_d = nc.dram_tensor("w2h", [128, CPC * HC], f16, kind="ExternalInput")
    out_d = nc.dram_tensor("out", [1, OUTCOLS], f32, kind="ExternalOutput")

    gelu = mybir.ActivationFunctionType.Gelu

    with TileContext(nc) as tc, ExitStack() as ctx:
        const = ctx.enter_context(tc.tile_pool(name="const", bufs=1))
        # Small constants on the SWDGE ring so they land before w1[0].
        ct = const.tile([KC, SELW + B1W], f16)
        nc.gpsimd.dma_start(out=ct[:], in_=cst_d[:])
        selt = ct[:, 0:SELW].rearrange("p (j t) -> p j t", j=KC)
        b1t = ct[:, SELW:SELW + B1W].rearrange("p (c g m) -> p c g m", c=CPC, g=2)
        w2t = const.tile([128, CPC * HC], f16)
        nc.gpsimd.dma_start(out=w2t[:], in_=w2_d[:])

        # Routed tokens on the ACT HWDGE ring (overlaps the SP ring's w1
        # stream): first slot lands early so the PE starts with w1[0].
        zall = const.tile([128, ZCOLS], f16)
        zs = zoff[1]
        nc.scalar.dma_start(out=zall[:, :zs], in_=zt_d[:, :zs])
        nc.scalar.dma_start(out=zall[:, zs:], in_=zt_d[:, zs:])

        # Staged per-company outputs; single store at the end (SWDGE).
        oall = const.tile([1, OUTCOLS], f32)

        # Per-company fp8 weights on the SP HWDGE ring, one DMA each: the
        # ring drains FIFO at full rate and compute pipelines behind it.
        w1p = ctx.enter_context(tc.tile_pool(name="w1p", bufs=1))
        w1ts = []
        for s in range(CPC):
            w1t = w1p.tile([128, 2, KC, H // 2], f8, name=f"w1_{s}")
            nc.sync.dma_start(out=w1t[:], in_=w1_d[s])
            w1ts.append(w1t)

        hp = ctx.enter_context(tc.tile_pool(name="hp", bufs=16))
        pp = ctx.enter_context(tc.tile_pool(name="pp", bufs=5, space="PSUM"))
        opp = ctx.enter_context(tc.tile_pool(name="opp", bufs=2, space="PSUM"))

        # PE warmup: keep the PE busy with dependency-free matmuls while
        # weights stream in so the HAM clock is warm (2.4GHz) for the
        # first real matmul.
        wsc = const.tile([128, 512], f16)
        nc.gpsimd.memset(wsc[:], 0.0)
        wps = ctx.enter_context(tc.tile_pool(name="wps", bufs=1, space="PSUM"))
        wp = wps.tile([128, 512], f32)
        for _ in range(22):
            nc.tensor.matmul(wp[:], wsc[:, :128], wsc[:], start=True, stop=True)

        for s in range(CPC):
            cap = caps[s]
            w1t = w1ts[s]
            for t0 in range(0, cap, 128):
                tw = min(128, cap - t0)
                osum_t = opp.tile([1, 128], f32, name="osum")
                osum = osum_t[:, :tw]
                for g in range(2):
                    ps_t = pp.tile([128, KC * 128], f32, name="ps")
                    ps = ps_t[:, :KC * tw]
                    # bias: ps[m, (j,t)] = 64*b1[c][512g+128j+m] via selector
                    nc.tensor.matmul(
                        ps, b1t[:, s, g, :], selt[:, :, :tw], start=True, stop=False
                    )
                    for j in range(KC):
                        zb = zoff[s] + t0
                        for k in range(KC):
                            nc.tensor.matmul(
                                ps[:, j * tw:(j + 1) * tw],
                                w1t[:, g, k, 128 * j:128 * (j + 1)],
                                zall[:, zb + k * cap: zb + k * cap + tw],
                                start=False,
                                stop=(k == KC - 1),
                            )
                    ht_t = hp.tile([128, KC * 128], f16, name="ht")
                    ht = ht_t[:, :KC * tw]
                    nc.scalar.activation(ht, ps, gelu, scale=1.0 / WSCALE)
                    for j in range(KC):
                        jj = KC * g + j
                        nc.tensor.matmul(
                            osum,
                            w2t[:, HC * s + jj:HC * s + jj + 1],
                            ht[:, j * tw:(j + 1) * tw],
                            start=(jj == 0),
                            stop=(jj == HC - 1),
                        )
                nc.vector.tensor_copy(oall[:, ooff[s] + t0: ooff[s] + t0 + tw], osum)

        osplit = ooff[max(1, CPC - 2)]
        nc.gpsimd.dma_start(out=out_d[:, :osplit], in_=oall[:, :osplit])
        nc.gpsimd.dma_start(out=out_d[:, osplit:], in_=oall[:, osplit:])

    nc.finalize()
    return nc


def _get_compiled(caps):
    key = tuple(caps)
    if key not in _COMPILED:
        _COMPILED[key] = _build(key)
    return _COMPILED[key]


def kernel(z, company_id, W1, b1, W2, b2):
    import ml_dtypes
    from concourse.bass_utils import run_bass_kernel_spmd

    f8np = ml_dtypes.float8_e3m4
    z = np.asarray(z, dtype=np.float32)
    cid = np.asarray(company_id).astype(np.int64).ravel()
    W1 = np.asarray(W1, dtype=np.float32)
    b1 = np.asarray(b1, dtype=np.float32)
    W2 = np.asarray(W2, dtype=np.float32)
    b2 = np.asarray(b2, dtype=np.float32)
    O = W2.shape[2]

    idx_by_company = [np.nonzero(cid == gc)[0] for gc in range(C)]
    cnt = np.array([len(ix) for ix in idx_by_company])
    order = np.argsort(-cnt, kind="stable")  # descending token count
    # slot s <- ranks [8s, 8s+8): core i takes order[8s+i]; shared capacity
    # is the group max rounded to 8.
    comp_at = [[int(order[CPC * s + core]) for s in range(CPC)] for core in range(NCORES)]
    caps = tuple(
        max(8, int(np.ceil(cnt[order[CPC * s]] / 8)) * 8) for s in range(CPC)
    )

    nc = _get_compiled(caps)

    zoffs = np.concatenate([[0], np.cumsum([KC * c for c in caps])])
    ooffs = np.concatenate([[0], np.cumsum(caps)])
    ZCOLS, OUTCOLS = int(zoffs[-1]), int(ooffs[-1])
    SELW = KC * 128
    B1W = CPC * 2 * 128
    sel = np.zeros((KC, KC, 128), dtype=np.float16)
    for j in range(KC):
        sel[j, j, :] = 1.0

    in_maps = []
    for core in range(NCORES):
        zt = np.zeros((128, ZCOLS), dtype=np.float16)
        w1 = np.zeros((CPC, 128, 2, KC, H // 2), dtype=f8np)
        b1h = np.zeros((KC, CPC, 2, 128), dtype=np.float16)
        w2h = np.zeros((128, CPC * HC), dtype=np.float16)
        for s in range(CPC):
            gc = comp_at[core][s]
            cap = caps[s]
            ix = idx_by_company[gc]
            if len(ix):
                zslot = np.zeros((cap, D), dtype=np.float16)
                zslot[:len(ix)] = z[ix].astype(np.float16)
                # [cap, (k,128)] -> [128, k, cap]
                zt[:, zoffs[s]:zoffs[s + 1]] = (
                    zslot.reshape(cap, KC, 128).transpose(2, 1, 0).reshape(128, KC * cap)
                )
            w1[s] = (
                np.clip(W1[gc] * WSCALE, -F8MAX, F8MAX)
                .reshape(KC, 128, 2, H // 2)
                .transpose(1, 2, 0, 3)
                .astype(f8np)
            )
            b1h[:, s] = (b1[gc] * WSCALE).reshape(2, KC, 128).transpose(1, 0, 2)
            w2h[:, HC * s:HC * (s + 1)] = (
                W2[gc, :, 0].reshape(HC, 128).T.astype(np.float16)
            )
        cst = np.zeros((KC, SELW + B1W), dtype=np.float16)
        cst[:, 0:SELW] = sel.reshape(KC, SELW)
        cst[:, SELW:] = b1h.reshape(KC, B1W)
        in_maps.append(
            {
                "zt": np.ascontiguousarray(zt),
                "w1": np.ascontiguousarray(w1),
                "cst": np.ascontiguousarray(cst),
                "w2h": np.ascontiguousarray(w2h),
            }
        )

    res = run_bass_kernel_spmd(nc, in_maps, list(range(NCORES)))

    out = np.zeros((B, O), dtype=np.float32)
    for core in range(NCORES):
        core_out = res.results[core]["out"].ravel()
        for s in range(CPC):
            gc = comp_at[core][s]
            ix = idx_by_company[gc]
            if len(ix):
                out[ix, 0] = core_out[ooffs[s]:ooffs[s] + len(ix)] + b2[gc, 0]
    return out


# revision 6
# speedup vs baseline: 1.0730x; 1.0730x over previous
"""Trainium2 Bass kernel for CompanySpecificHeads (MoE-style routed MLP heads).

Semantics (matching the reference):
    out[b] = gelu(z[b] @ W1[cid[b]] + b1[cid[b]]) @ W2[cid[b]] + b2[cid[b]]

Strategy: expert-parallel across 8 NeuronCores, 8 companies per core.
The run is DMA-bound on the W1 stream (HBM cap ~358 GB/s per core), so W1
is carried in fp8 E3M4 (scaled x64 on host; the x1/64 descale is folded
into the gelu activation's `scale` operand) while z / b1-selector / W2 /
gelu output stay fp16 -> the PE runs mixed fp8xfp16 matmuls (verified
numerically exact on HW). This halves the dominant DMA stream vs fp16.

Token capacity is variable per company slot: companies are sorted by
token count (descending) and slot s takes ranks [8s, 8s+8) across the 8
cores, so every core's slot-s company has a similar count and the shared
SPMD program pads only to the group max (~552 padded tokens/core vs 896
with a uniform capacity). Per company c (tokens on the free axis,
capacity tw, h on partitions):
      psum[h, t] = sum_d W1[c][d, h] * zT[c][d, t]     (PE, fp8 x fp16)
      b1*64 folded in with a K=4 fp16 "selector" matmul
  Gelu: ACT engine, psum*(1/64) -> gelu -> SBUF fp16.
  Layer 2: psum2[1, t] += W2[c][hj]^T @ gelu_h[hj, t]  (8 K=128 matmuls)
Host scatters back to [B, 1] and adds b2 (exact, fp32).

Trace-driven scheduling (v2):
- W1 is striped across BOTH HWDGE rings (even slots on SP, odd slots on
  the ACT ring behind cst/z) so the two packet streams together saturate
  the per-core HBM limit instead of one ring's ~82% packet efficiency.
- Layer 2 + output copy of slot s are emitted AFTER layer 1 of slot s+1:
  the PE never sits on the ACT engine's gelu latency (was ~9us of
  130-225ns stalls).
- Warmup is ~20 cheap N=128 matmuls ending near w1[0] arrival. The old
  22xN=512 warmup monopolized the FIFO PE queue for ~6us and pushed all
  real matmuls back by that much.
- Output stores go on the SP HWDGE ring (idle by then, ~0.6us first
  byte) instead of SWDGE (~2us fixed).
"""

import numpy as np

B, C, D, H = 4096, 64, 512, 1024
NCORES = 8
CPC = C // NCORES  # companies (slots) per core
KC = D // 128      # contraction chunks of 128
HC = H // 128      # h chunks of 128
WSCALE = 64.0      # W1 fp8 scale (descaled in the gelu activation)
F8MAX = 15.5       # e3m4 max finite

_COMPILED = {}


def _build(caps):
    """Build the Bass/Tile program for per-slot token capacities `caps`."""
    import concourse.bacc as bacc
    import concourse.mybir as mybir
    from concourse.tile import TileContext
    from contextlib import ExitStack

    f32 = mybir.dt.float32
    f16 = mybir.dt.float16
    f8 = mybir.dt.float8e3

    zoff = [0]
    for cap in caps:
        zoff.append(zoff[-1] + KC * cap)
    ZCOLS = zoff[-1]
    ooff = [0]
    for cap in caps:
        ooff.append(ooff[-1] + cap)
    OUTCOLS = ooff[-1]

    SELW = KC * 128          # selector columns (built at max subtile width)
    B1W = CPC * 2 * 128      # b1 columns

    nc = bacc.Bacc(None, target_bir_lowering=False)

    # zt is stored partition-major: zt[p, (s, k, t)] = z[token_t, 128k+p]
    zt_d = nc.dram_tensor("zt", [128, ZCOLS], f16, kind="ExternalInput")
    # w1[s][p][g][k][hh] = W1[comp(s)][128k+p, 512g+hh] * 64, fp8: a whole
    # company loads linearly with 4KB contiguous per partition.
    w1_d = nc.dram_tensor("w1", [CPC, 128, 2, KC, H // 2], f8, kind="ExternalInput")
    cst_d = nc.dram_tensor("cst", [KC, SELW + B1W], f16, kind="ExternalInput")
    w2_d = nc.dram_tensor("w2h", [128, CPC * HC], f16, kind="ExternalInput")
    out_d = nc.dram_tensor("out", [1, OUTCOLS], f32, kind="ExternalOutput")

    gelu = mybir.ActivationFunctionType.Gelu

    with TileContext(nc) as tc, ExitStack() as ctx:
        const = ctx.enter_context(tc.tile_pool(name="const", bufs=1))
        # Constants lead the ACT HWDGE ring: they gate the first bias
        # matmul, and SWDGE would deliver them ~2us later.
        ct = const.tile([KC, SELW + B1W], f16)
        nc.scalar.dma_start(out=ct[:], in_=cst_d[:])
        selt = ct[:, 0:SELW].rearrange("p (j t) -> p j t", j=KC)
        b1t = ct[:, SELW:SELW + B1W].rearrange("p (c g m) -> p c g m", c=CPC, g=2)
        w2t = const.tile([128, CPC * HC], f16)
        nc.gpsimd.dma_start(out=w2t[:], in_=w2_d[:])

        zall = const.tile([128, ZCOLS], f16)
        w1p = ctx.enter_context(tc.tile_pool(name="w1p", bufs=1))
        w1ts = [
            w1p.tile([128, 2, KC, H // 2], f8, name=f"w1_{s}") for s in range(CPC)
        ]
        # Interleave z and W1 on the ACT ring in the order compute needs
        # them; even W1 slots stream on the SP ring concurrently. Both
        # rings together track the per-core HBM limit.
        nc.scalar.dma_start(out=zall[:, :zoff[1]], in_=zt_d[:, :zoff[1]])
        nc.sync.dma_start(out=w1ts[0][:], in_=w1_d[0])
        nc.scalar.dma_start(out=w1ts[1][:], in_=w1_d[1])
        nc.sync.dma_start(out=w1ts[2][:], in_=w1_d[2])
        nc.scalar.dma_start(out=zall[:, zoff[1]:zoff[4]], in_=zt_d[:, zoff[1]:zoff[4]])
        nc.scalar.dma_start(out=w1ts[3][:], in_=w1_d[3])
        nc.sync.dma_start(out=w1ts[4][:], in_=w1_d[4])
        nc.scalar.dma_start(out=zall[:, zoff[4]:], in_=zt_d[:, zoff[4]:])
        nc.scalar.dma_start(out=w1ts[5][:], in_=w1_d[5])
        nc.sync.dma_start(out=w1ts[6][:], in_=w1_d[6])
        nc.scalar.dma_start(out=w1ts[7][:], in_=w1_d[7])

        # Staged per-company outputs; stored at the end on the idle SP ring.
        oall = const.tile([1, OUTCOLS], f32)

        hp = ctx.enter_context(tc.tile_pool(name="hp", bufs=16))
        pp = ctx.enter_context(tc.tile_pool(name="pp", bufs=5, space="PSUM"))
        opp = ctx.enter_context(tc.tile_pool(name="opp", bufs=2, space="PSUM"))

        # PE warmup: ~20 cheap matmuls keep the PE's HAM activity window
        # busy from the end of the framework preamble (~6us) until w1[0]
        # lands (~10us) so real matmuls start at the warm 2.4GHz clock.
        # Cheap (N=128, one weight set) so the FIFO PE queue drains the
        # moment real work is ready.
        wsc = const.tile([128, 128], f16)
        nc.gpsimd.memset(wsc[:], 0.0)
        wps = ctx.enter_context(tc.tile_pool(name="wps", bufs=1, space="PSUM"))
        wp = wps.tile([128, 128], f32)
        for _ in range(20):
            nc.tensor.matmul(wp[:], wsc[:], wsc[:], start=True, stop=True)

        # Software-pipelined main loop: layer 2 of iteration i-1 is
        # emitted after layer 1 of iteration i, so the PE never waits on
        # the gelu. PSUM: 2 ps banks/iter x 2 iters in flight + osum.
        pending = None  # (s, t0, tw, [ht_g0, ht_g1])

        def emit_layer2(item):
            s, t0, tw, hts = item
            osum_t = opp.tile([1, 128], f32, name="osum")
            osum = osum_t[:, :tw]
            for g in range(2):
                for j in range(KC):
                    jj = KC * g + j
                    nc.tensor.matmul(
                        osum,
                        w2t[:, HC * s + jj:HC * s + jj + 1],
                        hts[g][:, j * tw:(j + 1) * tw],
                        start=(jj == 0),
                        stop=(jj == HC - 1),
                    )
            nc.vector.tensor_copy(oall[:, ooff[s] + t0: ooff[s] + t0 + tw], osum)

        for s in range(CPC):
            cap = caps[s]
            w1t = w1ts[s]
            for t0 in range(0, cap, 128):
                tw = min(128, cap - t0)
                hts = []
                for g in range(2):
                    ps_t = pp.tile([128, KC * 128], f32, name="ps")
                    ps = ps_t[:, :KC * tw]
                    # bias: ps[m, (j,t)] = 64*b1[c][512g+128j+m] via selector
                    nc.tensor.matmul(
                        ps, b1t[:, s, g, :], selt[:, :, :tw], start=True, stop=False
                    )
                    zb = zoff[s] + t0
                    for j in range(KC):
                        for k in range(KC):
                            nc.tensor.matmul(
                                ps[:, j * tw:(j + 1) * tw],
                                w1t[:, g, k, 128 * j:128 * (j + 1)],
                                zall[:, zb + k * cap: zb + k * cap + tw],
                                start=False,
                                stop=(k == KC - 1),
                            )
                    ht_t = hp.tile([128, KC * 128], f16, name="ht")
                    ht = ht_t[:, :KC * tw]
                    nc.scalar.activation(ht, ps, gelu, scale=1.0 / WSCALE)
                    hts.append(ht)
                if pending is not None:
                    emit_layer2(pending)
                pending = (s, t0, tw, hts)
        emit_layer2(pending)

        osplit = ooff[max(1, CPC - 2)]
        nc.sync.dma_start(out=out_d[:, :osplit], in_=oall[:, :osplit])
        nc.sync.dma_start(out=out_d[:, osplit:], in_=oall[:, osplit:])

    nc.finalize()
    return nc


def _get_compiled(caps):
    key = tuple(caps)
    if key not in _COMPILED:
        _COMPILED[key] = _build(key)
    return _COMPILED[key]


def kernel(z, company_id, W1, b1, W2, b2):
    import ml_dtypes
    from concourse.bass_utils import run_bass_kernel_spmd

    f8np = ml_dtypes.float8_e3m4
    z = np.asarray(z, dtype=np.float32)
    cid = np.asarray(company_id).astype(np.int64).ravel()
    W1 = np.asarray(W1, dtype=np.float32)
    b1 = np.asarray(b1, dtype=np.float32)
    W2 = np.asarray(W2, dtype=np.float32)
    b2 = np.asarray(b2, dtype=np.float32)
    O = W2.shape[2]

    idx_by_company = [np.nonzero(cid == gc)[0] for gc in range(C)]
    cnt = np.array([len(ix) for ix in idx_by_company])
    order = np.argsort(-cnt, kind="stable")  # descending token count
    # slot s <- ranks [8s, 8s+8): core i takes order[8s+i]; shared capacity
    # is the group max rounded to 8.
    comp_at = [[int(order[CPC * s + core]) for s in range(CPC)] for core in range(NCORES)]
    caps = tuple(
        max(8, int(np.ceil(cnt[order[CPC * s]] / 8)) * 8) for s in range(CPC)
    )

    nc = _get_compiled(caps)

    zoffs = np.concatenate([[0], np.cumsum([KC * c for c in caps])])
    ooffs = np.concatenate([[0], np.cumsum(caps)])
    ZCOLS, OUTCOLS = int(zoffs[-1]), int(ooffs[-1])
    SELW = KC * 128
    B1W = CPC * 2 * 128
    sel = np.zeros((KC, KC, 128), dtype=np.float16)
    for j in range(KC):
        sel[j, j, :] = 1.0

    in_maps = []
    for core in range(NCORES):
        zt = np.zeros((128, ZCOLS), dtype=np.float16)
        w1 = np.zeros((CPC, 128, 2, KC, H // 2), dtype=f8np)
        b1h = np.zeros((KC, CPC, 2, 128), dtype=np.float16)
        w2h = np.zeros((128, CPC * HC), dtype=np.float16)
        for s in range(CPC):
            gc = comp_at[core][s]
            cap = caps[s]
            ix = idx_by_company[gc]
            if len(ix):
                zslot = np.zeros((cap, D), dtype=np.float16)
                zslot[:len(ix)] = z[ix].astype(np.float16)
                # [cap, (k,128)] -> [128, k, cap]
                zt[:, zoffs[s]:zoffs[s + 1]] = (
                    zslot.reshape(cap, KC, 128).transpose(2, 1, 0).reshape(128, KC * cap)
                )
            w1[s] = (
                np.clip(W1[gc] * WSCALE, -F8MAX, F8MAX)
                .reshape(KC, 128, 2, H // 2)
                .transpose(1, 2, 0, 3)
                .astype(f8np)
            )
            b1h[:, s] = (b1[gc] * WSCALE).reshape(2, KC, 128).transpose(1, 0, 2)
            w2h[:, HC * s:HC * (s + 1)] = (
                W2[gc, :, 0].reshape(HC, 128).T.astype(np.float16)
            )
        cst = np.zeros((KC, SELW + B1W), dtype=np.float16)
        cst[:, 0:SELW] = sel.reshape(KC, SELW)
        cst[:, SELW:] = b1h.reshape(KC, B1W)
        in_maps.append(
            {
                "zt": np.ascontiguousarray(zt),
                "w1": np.ascontiguousarray(w1),
                "cst": np.ascontiguousarray(cst),
                "w2h": np.ascontiguousarray(w2h),
            }
        )

    res = run_bass_kernel_spmd(nc, in_maps, list(range(NCORES)))

    out = np.zeros((B, O), dtype=np.float32)
    for core in range(NCORES):
        core_out = res.results[core]["out"].ravel()
        for s in range(CPC):
            gc = comp_at[core][s]
            ix = idx_by_company[gc]
            if len(ix):
                out[ix, 0] = core_out[ooffs[s]:ooffs[s] + len(ix)] + b2[gc, 0]
    return out


# revision 7
# speedup vs baseline: 1.1005x; 1.0257x over previous
"""Trainium2 Bass kernel for CompanySpecificHeads (MoE-style routed MLP heads).

Semantics (matching the reference):
    out[b] = gelu(z[b] @ W1[cid[b]] + b1[cid[b]]) @ W2[cid[b]] + b2[cid[b]]

Strategy: expert-parallel across 8 NeuronCores, 8 companies per core.
The run is DMA-bound on the W1 stream (HBM cap ~358 GB/s per core), so W1
is carried in fp8 E3M4 (scaled x64 on host; the x1/64 descale is folded
into the gelu activation's `scale` operand) while z / b1-selector / W2 /
gelu output stay fp16 -> the PE runs mixed fp8xfp16 matmuls (verified
numerically exact on HW). This halves the dominant DMA stream vs fp16.

Token capacity is variable per company slot: companies are sorted by
token count (descending) and slot s takes ranks [8s, 8s+8) across the 8
cores, so every core's slot-s company has a similar count and the shared
SPMD program pads only to the group max (~552 padded tokens/core vs 896
with a uniform capacity). Per company c (tokens on the free axis,
capacity tw, h on partitions):
      psum[h, t] = sum_d W1[c][d, h] * zT[c][d, t]     (PE, fp8 x fp16)
      b1*64 folded in with a K=4 fp16 "selector" matmul
  Gelu: ACT engine, psum*(1/64) -> gelu -> SBUF fp16.
  Layer 2: psum2[1, t] += W2[c][hj]^T @ gelu_h[hj, t]  (8 K=128 matmuls)
Host scatters back to [B, 1] and adds b2 (exact, fp32).

Trace-driven scheduling (v2):
- W1 is striped across BOTH HWDGE rings (even slots on SP, odd slots on
  the ACT ring behind cst/z) so the two packet streams together saturate
  the per-core HBM limit instead of one ring's ~82% packet efficiency.
- Layer 2 + output copy of slot s are emitted AFTER layer 1 of slot s+1:
  the PE never sits on the ACT engine's gelu latency (was ~9us of
  130-225ns stalls).
- Warmup is ~20 cheap N=128 matmuls ending near w1[0] arrival. The old
  22xN=512 warmup monopolized the FIFO PE queue for ~6us and pushed all
  real matmuls back by that much.
- Output stores go on the SP HWDGE ring (idle by then, ~0.6us first
  byte) instead of SWDGE (~2us fixed).
"""

import numpy as np

B, C, D, H = 4096, 64, 512, 1024
NCORES = 8
CPC = C // NCORES  # companies (slots) per core
KC = D // 128      # contraction chunks of 128
HC = H // 128      # h chunks of 128
WSCALE = 64.0      # W1 fp8 scale (descaled in the gelu activation)
F8MAX = 15.5       # e3m4 max finite

_COMPILED = {}


def _build(caps):
    """Build the Bass/Tile program for per-slot token capacities `caps`."""
    import concourse.bacc as bacc
    import concourse.mybir as mybir
    from concourse.tile import TileContext
    from contextlib import ExitStack

    f32 = mybir.dt.float32
    f16 = mybir.dt.float16
    f8 = mybir.dt.float8e3

    zoff = [0]
    for cap in caps:
        zoff.append(zoff[-1] + KC * cap)
    ZCOLS = zoff[-1]
    ooff = [0]
    for cap in caps:
        ooff.append(ooff[-1] + cap)
    OUTCOLS = ooff[-1]

    SELW = KC * 128          # selector columns (built at max subtile width)
    B1W = CPC * 2 * 128      # b1 columns

    nc = bacc.Bacc(None, target_bir_lowering=False)

    # zt is stored partition-major: zt[p, (s, k, t)] = z[token_t, 128k+p]
    zt_d = nc.dram_tensor("zt", [128, ZCOLS], f16, kind="ExternalInput")
    # w1[s][p][g][k][hh] = W1[comp(s)][128k+p, 512g+hh] * 64, fp8: a whole
    # company loads linearly with 4KB contiguous per partition.
    w1_d = nc.dram_tensor("w1", [CPC, 128, 2, KC, H // 2], f8, kind="ExternalInput")
    cst_d = nc.dram_tensor("cst", [KC, SELW + B1W], f16, kind="ExternalInput")
    w2_d = nc.dram_tensor("w2h", [128, CPC * HC], f16, kind="ExternalInput")
    out_d = nc.dram_tensor("out", [1, OUTCOLS], f32, kind="ExternalOutput")

    gelu = mybir.ActivationFunctionType.Gelu

    with TileContext(nc) as tc, ExitStack() as ctx:
        const = ctx.enter_context(tc.tile_pool(name="const", bufs=1))
        # Constants lead the ACT HWDGE ring: they gate the first bias
        # matmul, and SWDGE would deliver them ~2us later.
        ct = const.tile([KC, SELW + B1W], f16)
        nc.sync.dma_start(out=ct[:], in_=cst_d[:])
        selt = ct[:, 0:SELW].rearrange("p (j t) -> p j t", j=KC)
        b1t = ct[:, SELW:SELW + B1W].rearrange("p (c g m) -> p c g m", c=CPC, g=2)
        w2t = const.tile([128, CPC * HC], f16)
        nc.gpsimd.dma_start(out=w2t[:], in_=w2_d[:])

        zall = const.tile([128, ZCOLS], f16)
        w1p = ctx.enter_context(tc.tile_pool(name="w1p", bufs=1))
        w1ts = [
            w1p.tile([128, 2, KC, H // 2], f8, name=f"w1_{s}") for s in range(CPC)
        ]
        # Interleave z and W1 on the ACT ring in the order compute needs
        # them; even W1 slots stream on the SP ring concurrently. Both
        # rings together track the per-core HBM limit.
        nc.sync.dma_start(out=zall[:, :zoff[1]], in_=zt_d[:, :zoff[1]])
        nc.sync.dma_start(out=w1ts[0][:], in_=w1_d[0])
        nc.scalar.dma_start(out=w1ts[1][:], in_=w1_d[1])
        nc.sync.dma_start(out=w1ts[2][:], in_=w1_d[2])
        nc.scalar.dma_start(out=zall[:, zoff[1]:zoff[4]], in_=zt_d[:, zoff[1]:zoff[4]])
        nc.scalar.dma_start(out=w1ts[3][:], in_=w1_d[3])
        nc.sync.dma_start(out=w1ts[4][:], in_=w1_d[4])
        nc.scalar.dma_start(out=zall[:, zoff[4]:], in_=zt_d[:, zoff[4]:])
        nc.scalar.dma_start(out=w1ts[5][:], in_=w1_d[5])
        nc.sync.dma_start(out=w1ts[6][:], in_=w1_d[6])
        nc.scalar.dma_start(out=w1ts[7][:], in_=w1_d[7])

        # Staged per-company outputs; stored at the end on the idle SP ring.
        oall = const.tile([1, OUTCOLS], f32)

        hp = ctx.enter_context(tc.tile_pool(name="hp", bufs=16))
        pp = ctx.enter_context(tc.tile_pool(name="pp", bufs=5, space="PSUM"))
        opp = ctx.enter_context(tc.tile_pool(name="opp", bufs=2, space="PSUM"))

        # PE warmup: ~20 cheap matmuls keep the PE's HAM activity window
        # busy from the end of the framework preamble (~6us) until w1[0]
        # lands (~10us) so real matmuls start at the warm 2.4GHz clock.
        # Cheap (N=128, one weight set) so the FIFO PE queue drains the
        # moment real work is ready.
        wsc = const.tile([128, 128], f16)
        nc.gpsimd.memset(wsc[:], 0.0)
        wps = ctx.enter_context(tc.tile_pool(name="wps", bufs=1, space="PSUM"))
        wp = wps.tile([128, 128], f32)
        for _ in range(20):
            nc.tensor.matmul(wp[:], wsc[:], wsc[:], start=True, stop=True)

        # Software-pipelined main loop: layer 2 of iteration i-1 is
        # emitted after layer 1 of iteration i, so the PE never waits on
        # the gelu. PSUM: 2 ps banks/iter x 2 iters in flight + osum.
        pending = None  # (s, t0, tw, [ht_g0, ht_g1])

        def emit_layer2(item):
            s, t0, tw, hts = item
            osum_t = opp.tile([1, 128], f32, name="osum")
            osum = osum_t[:, :tw]
            for g in range(2):
                for j in range(KC):
                    jj = KC * g + j
                    nc.tensor.matmul(
                        osum,
                        w2t[:, HC * s + jj:HC * s + jj + 1],
                        hts[g][:, j * tw:(j + 1) * tw],
                        start=(jj == 0),
                        stop=(jj == HC - 1),
                    )
            nc.vector.tensor_copy(oall[:, ooff[s] + t0: ooff[s] + t0 + tw], osum)

        for s in range(CPC):
            cap = caps[s]
            w1t = w1ts[s]
            for t0 in range(0, cap, 128):
                tw = min(128, cap - t0)
                hts = []
                pss = []
                for g in range(2):
                    ps_t = pp.tile([128, KC * 128], f32, name="ps")
                    ps = ps_t[:, :KC * tw]
                    # bias: ps[m, (j,t)] = 64*b1[c][512g+128j+m] via selector
                    nc.tensor.matmul(
                        ps, b1t[:, s, g, :], selt[:, :, :tw], start=True, stop=False
                    )
                    pss.append(ps)
                for g in range(2):
                    ps = pss[g]
                    zb = zoff[s] + t0
                    for j in range(KC):
                        for k in range(KC):
                            nc.tensor.matmul(
                                ps[:, j * tw:(j + 1) * tw],
                                w1t[:, g, k, 128 * j:128 * (j + 1)],
                                zall[:, zb + k * cap: zb + k * cap + tw],
                                start=False,
                                stop=(k == KC - 1),
                            )
                    ht_t = hp.tile([128, KC * 128], f16, name="ht")
                    ht = ht_t[:, :KC * tw]
                    nc.scalar.activation(ht, ps, gelu, scale=1.0 / WSCALE)
                    hts.append(ht)
                if pending is not None:
                    emit_layer2(pending)
                pending = (s, t0, tw, hts)
        emit_layer2(pending)

        osplit = ooff[max(1, CPC - 2)]
        olast = ooff[max(1, CPC - 1)]
        nc.sync.dma_start(out=out_d[:, :osplit], in_=oall[:, :osplit])
        nc.sync.dma_start(out=out_d[:, osplit:olast], in_=oall[:, osplit:olast])
        nc.scalar.dma_start(out=out_d[:, olast:], in_=oall[:, olast:])

    nc.finalize()
    return nc


def _get_compiled(caps):
    key = tuple(caps)
    if key not in _COMPILED:
        _COMPILED[key] = _build(key)
    return _COMPILED[key]


def kernel(z, company_id, W1, b1, W2, b2):
    import ml_dtypes
    from concourse.bass_utils import run_bass_kernel_spmd

    f8np = ml_dtypes.float8_e3m4
    z = np.asarray(z, dtype=np.float32)
    cid = np.asarray(company_id).astype(np.int64).ravel()
    W1 = np.asarray(W1, dtype=np.float32)
    b1 = np.asarray(b1, dtype=np.float32)
    W2 = np.asarray(W2, dtype=np.float32)
    b2 = np.asarray(b2, dtype=np.float32)
    O = W2.shape[2]

    idx_by_company = [np.nonzero(cid == gc)[0] for gc in range(C)]
    cnt = np.array([len(ix) for ix in idx_by_company])
    order = np.argsort(-cnt, kind="stable")  # descending token count
    # slot s <- ranks [8s, 8s+8): core i takes order[8s+i]; shared capacity
    # is the group max rounded to 8.
    comp_at = [[int(order[CPC * s + core]) for s in range(CPC)] for core in range(NCORES)]
    caps = tuple(
        max(8, int(np.ceil(cnt[order[CPC * s]] / 8)) * 8) for s in range(CPC)
    )

    nc = _get_compiled(caps)

    zoffs = np.concatenate([[0], np.cumsum([KC * c for c in caps])])
    ooffs = np.concatenate([[0], np.cumsum(caps)])
    ZCOLS, OUTCOLS = int(zoffs[-1]), int(ooffs[-1])
    SELW = KC * 128
    B1W = CPC * 2 * 128
    sel = np.zeros((KC, KC, 128), dtype=np.float16)
    for j in range(KC):
        sel[j, j, :] = 1.0

    in_maps = []
    for core in range(NCORES):
        zt = np.zeros((128, ZCOLS), dtype=np.float16)
        w1 = np.zeros((CPC, 128, 2, KC, H // 2), dtype=f8np)
        b1h = np.zeros((KC, CPC, 2, 128), dtype=np.float16)
        w2h = np.zeros((128, CPC * HC), dtype=np.float16)
        for s in range(CPC):
            gc = comp_at[core][s]
            cap = caps[s]
            ix = idx_by_company[gc]
            if len(ix):
                zslot = np.zeros((cap, D), dtype=np.float16)
                zslot[:len(ix)] = z[ix].astype(np.float16)
                # [cap, (k,128)] -> [128, k, cap]
                zt[:, zoffs[s]:zoffs[s + 1]] = (
                    zslot.reshape(cap, KC, 128).transpose(2, 1, 0).reshape(128, KC * cap)
                )
            w1[s] = (
                np.clip(W1[gc] * WSCALE, -F8MAX, F8MAX)
                .reshape(KC, 128, 2, H // 2)
                .transpose(1, 2, 0, 3)
                .astype(f8np)
            )
            b1h[:, s] = (b1[gc] * WSCALE).reshape(2, KC, 128).transpose(1, 0, 2)
            w2h[:, HC * s:HC * (s + 1)] = (
                W2[gc, :, 0].reshape(HC, 128).T.astype(np.float16)
            )
        cst = np.zeros((KC, SELW + B1W), dtype=np.float16)
        cst[:, 0:SELW] = sel.reshape(KC, SELW)
        cst[:, SELW:] = b1h.reshape(KC, B1W)
        in_maps.append(
            {
                "zt": np.ascontiguousarray(zt),
                "w1": np.ascontiguousarray(w1),
                "cst": np.ascontiguousarray(cst),
                "w2h": np.ascontiguousarray(w2h),
            }
        )

    res = run_bass_kernel_spmd(nc, in_maps, list(range(NCORES)))

    out = np.zeros((B, O), dtype=np.float32)
    for core in range(NCORES):
        core_out = res.results[core]["out"].ravel()
        for s in range(CPC):
            gc = comp_at[core][s]
            ix = idx_by_company[gc]
            if len(ix):
                out[ix, 0] = core_out[ooffs[s]:ooffs[s] + len(ix)] + b2[gc, 0]
    return out
